# revision 46
# baseline (speedup 1.0000x reference)
"""Hawk (RG-LRU recurrent block) Trainium2 kernel, 8-core SPMD.

Sharding: data-parallel over B (2 groups of 4 cores) x sequence-parallel over T
(4 chunks of 1024 tokens per batch element). The diagonal linear recurrence
h_t = a_t * h_{t-1} + b_t is computed chunk-locally with the hardware
tensor_tensor_scan instruction, then stitched across cores with AllGathers of
per-chunk scan summaries (A = prod a, b = local final h) and the per-core
correction h = h_local + cumprod(a) * carry.

Layout on device: hidden channels on partitions, time on the free dimension.
All matmuls run in bf16 (both operands); the recurrence (alpha/beta/scan)
stays fp32. Weights are fully pre-transposed on the host so every weight DMA
is a dense contiguous copy, and the weight pool is deep enough to prefetch
across phase boundaries. The cumprod tiles stay SBUF-resident in bf16 (the
scan state is fp32 internally regardless of output dtype).

Key scheduling structure:
- sigmoid is computed as 0.5 + 0.5*tanh(x/2): Tanh/Exp/Square live in ONE
  activation-table set, so the per-channel chain costs 2 table loads
  (exp-set, sqrt-set) instead of 3.
- the summary AllGather is split in two (channels 0-8 mid-phase-2, 9-11 at
  the end) so most carry corrections overlap the gate matmuls.
- the correction is folded into the output operand as
  gh = (gg*hl) + (gg*P)*c: the two products need no carry and pipeline
  early; only one scalar_tensor_tensor per channel remains after the
  gather.
- the gate-half projection writes two half-width PSUM tiles from a separate
  pool, decoupling it from the gate-matmul PSUM ring.

Per core:
  phase 1: proj (u half) from xT -> u_pre -> causal depthwise conv -> u_c
           + 4 gate-half proj iterations to cover the conv's DVE tail
  phase 2: ig/rg gate matmuls -> tanh -> alpha/xbeta -> scans (h_loc, P),
           with the first summary AllGather after channel chunk 8
  phase 2.5: remaining 8 gate-half proj + gelu + gg*hl / gg*P products
  phase 3: per-channel carry fold + output projection
"""
import os

os.environ.setdefault("JAX_COMPILATION_CACHE_DIR", "/tmp/jax_cache_hawk")

import ml_dtypes
import numpy as np

import concourse.bacc as bacc
import concourse.mybir as mybir
import concourse.tile as tile
from concourse.bass_utils import run_bass_kernel_spmd

F32 = mybir.dt.float32
BF16 = mybir.dt.bfloat16
AF = mybir.ActivationFunctionType
OP = mybir.AluOpType
NPBF16 = ml_dtypes.bfloat16

DIM = 1024
HID = 1536
KCONV = 4
B = 2
T = 4096
C_CONST = 8.0

NCORE = 8
TC = T // 4          # tokens per core
NH = HID // 128      # 12 hidden chunks
NDC = DIM // 128     # 8 dim chunks
PADL = 4             # left pad columns (3 conv-overlap tokens + 1 alignment)
TPAD = TC + PADL     # 1028
GV_BASE = NH * 5     # packed consts: conv(60) | hn,hbi,hbg(36) | sel(3)
SEL_BASE = GV_BASE + NH * 3
NCC = SEL_BASE + 3

N25_EARLY = 4        # gate-proj iterations covering the phase-1 conv tail

_CACHE: dict = {}


def _build():
    nc = bacc.Bacc("TRN2", target_bir_lowering=False, debug=False,
                   num_devices=NCORE, dynamic_dma_scratch_size=8192)

    xt = nc.dram_tensor("xt", [NDC, 128, TPAD], BF16, kind="ExternalInput").ap()
    wproj = nc.dram_tensor("wproj", [24, 128, NDC * 128], BF16,
                           kind="ExternalInput").ap()
    wgates = nc.dram_tensor("wgates", [2, NH, 128, NH * 128], BF16,
                            kind="ExternalInput").ap()
    wout = nc.dram_tensor("wout", [NDC, 128, NH * 128], BF16,
                          kind="ExternalInput").ap()
    consts = nc.dram_tensor("consts", [128, NCC], F32, kind="ExternalInput").ap()
    out = nc.dram_tensor("out", [NDC, 128, TC], F32, kind="ExternalOutput").ap()

    with tile.TileContext(nc) as tc:
        with (
            tc.tile_pool(name="cst", bufs=1) as cst,
            tc.tile_pool(name="wp", bufs=10) as wp,
            tc.tile_pool(name="pxt", bufs=8) as pxt,
            tc.tile_pool(name="puc", bufs=12) as puc,
            tc.tile_pool(name="phl", bufs=12) as phl,
            tc.tile_pool(name="ppt", bufs=12) as ppt,
            tc.tile_pool(name="pgg", bufs=12) as pgg,
            tc.tile_pool(name="rf", bufs=5) as rf,
            tc.tile_pool(name="rb", bufs=6) as rb,
            tc.tile_pool(name="pb2", bufs=2) as pb2,
            tc.tile_pool(name="pwo", bufs=2) as pwo,
            tc.tile_pool(name="ps", bufs=3, space="PSUM") as ps,
            tc.tile_pool(name="psb", bufs=2, space="PSUM") as psb,
            tc.tile_pool(name="dram", bufs=1, space="DRAM") as dram,
        ):
            def load_w(src, ncols):
                wt = wp.tile([128, NH * 128], BF16, tag="w", name="wt")
                nc.sync.dma_start(wt[:, 0:ncols], src)
                return wt

            # first weight + x resident (emission order = DMA issue order)
            w_first = load_w(wproj[0], NDC * 128)
            xt_t = []
            for cc in range(NDC):
                x1 = pxt.tile([128, TPAD], BF16, tag="xt", name="x1")
                nc.sync.dma_start(x1[:], xt[cc])
                xt_t.append(x1)
            cs = cst.tile([128, NCC], F32, tag="cs", name="cs")
            nc.sync.dma_start(cs[:], consts[:])

            def cw(hc, k):
                return cs[:, hc * 5 + k: hc * 5 + k + 1]

            def gv(hc, k):
                return cs[:, GV_BASE + hc * 3 + k: GV_BASE + hc * 3 + k + 1]

            sel_t = cs[:, SEL_BASE: SEL_BASE + 3]

            S_loc = cst.tile([128, 24], F32, tag="sloc", name="S_loc")
            p2_t = cst.tile([128, NH], F32, tag="p2", name="p2_t")
            p3_t = cst.tile([128, NH], F32, tag="p3", name="p3_t")
            c_t = cst.tile([128, NH], F32, tag="ct", name="c_t")

            def proj_matmuls(pt, wt, prefix_pt, rhs_tiles=None):
                # prefix matmuls are interleaved per contraction chunk: a
                # standalone block of tiny N=4 matmuls would serialize on
                # unhidden LDWEIGHTS (not modeled, but real); sandwiched
                # between the 512-wide matmuls the loads overlap streaming
                src = rhs_tiles or xt_t
                for cc in range(NDC):
                    wslice = wt[:, cc * 128:(cc + 1) * 128]
                    for th in range(2):
                        nc.tensor.matmul(
                            pt[:, th * 512:(th + 1) * 512],
                            wslice,
                            src[cc][:, PADL + th * 512: PADL + (th + 1) * 512],
                            start=(cc == 0), stop=(cc == NDC - 1))
                    if prefix_pt is not None:
                        nc.tensor.matmul(
                            prefix_pt[:, 0:PADL],
                            wslice,
                            src[cc][:, 0:PADL],
                            start=(cc == 0), stop=(cc == NDC - 1))

            # ---- phase 1: u half of proj + causal conv ----
            u_c = []
            for mc in range(NH):
                wt = w_first if mc == 0 else load_w(wproj[mc], NDC * 128)
                pt = ps.tile([128, TC], F32, tag="ps", name="pt")
                p3 = psb.tile([128, 512], F32, tag="psb", name="p3")
                proj_matmuls(pt, wt, p3)
                upre = rf.tile([128, TPAD], F32, tag="rf", name="upre")
                nc.scalar.copy(upre[:, PADL:TPAD], pt[:])
                nc.vector.tensor_copy(upre[:, 0:PADL], p3[:, 0:PADL])
                acc = rf.tile([128, TPAD], F32, tag="rf", name="acc")
                # u_c[t] = sum_k w_k * u_pre[t-3+k] + conv_b; token t at col
                # PADL+t, so tap k reads cols [1+k : 1+k+TC)
                nc.vector.tensor_scalar(
                    acc[:, 0:TC], upre[:, 1:1 + TC], cw(mc, 0), cw(mc, 4),
                    OP.mult, OP.add)
                for k in (1, 2):
                    nc.vector.scalar_tensor_tensor(
                        acc[:, 0:TC], upre[:, 1 + k:1 + k + TC], cw(mc, k),
                        acc[:, 0:TC], OP.mult, OP.add)
                uc = puc.tile([128, TC], BF16, tag="uc", name="uc")
                nc.vector.scalar_tensor_tensor(
                    uc[:], upre[:, PADL:PADL + TC], cw(mc, 3),
                    acc[:, 0:TC], OP.mult, OP.add)
                u_c.append(uc)

            # ---- gate half of proj + gelu (two half-width PSUM tiles from
            # the psb pool so these matmuls never contend with the ps ring) --
            gg_t: list = [None] * NH

            def gate_proj(i):
                wt = load_w(wproj[NH + i], NDC * 128)
                gg = pgg.tile([128, TC], BF16, tag="gg", name="gg")
                phs = [psb.tile([128, 512], F32, tag="psb", name="ph")
                       for _ in range(2)]
                for cc in range(NDC):
                    for th in range(2):
                        nc.tensor.matmul(
                            phs[th][:, 0:512],
                            wt[:, cc * 128:(cc + 1) * 128],
                            xt_t[cc][:, PADL + th * 512: PADL + (th + 1) * 512],
                            start=(cc == 0), stop=(cc == NDC - 1))
                for th in range(2):
                    nc.scalar.activation(gg[:, th * 512:(th + 1) * 512],
                                         phs[th][:, 0:512], AF.Gelu)
                gg_t[i] = gg

            for i in range(N25_EARLY):
                gate_proj(i)

            # ---- phase 2: gates + scans, in batches ----
            # sigmoid(z) = 0.5 + 0.5*tanh(z/2), so alpha folds into one Exp:
            # alpha = exp(negrate*sig(rg)) = exp(hn*tanh_rg + hn), hn = negrate/2
            # and xbeta = beta*sig(ig)*u = sqrt(0.25*(1-alpha^2+eps)) *
            #             ((tanh_ig + 1)*u)
            h_loc: list = [None] * NH
            p_t: list = [None] * NH

            def emit_2a(gcs, A_t, X_t):
                for gc in gcs:
                    pig = ps.tile([128, TC], F32, tag="ps", name="pig")
                    prg = ps.tile([128, TC], F32, tag="ps", name="prg")
                    for dst, wi in ((pig, 0), (prg, 1)):
                        wt = load_w(wgates[wi, gc], NH * 128)
                        # hc-outer so each weight block feeds back-to-back
                        # matmuls (halves real LDWEIGHTS traffic)
                        for hc in range(NH):
                            for th in range(2):
                                nc.tensor.matmul(
                                    dst[:, th * 512:(th + 1) * 512],
                                    wt[:, hc * 128:(hc + 1) * 128],
                                    u_c[hc][:, th * 512:(th + 1) * 512],
                                    start=(hc == 0), stop=(hc == NH - 1))
                    at = rf.tile([128, TPAD], F32, tag="rf", name="at")
                    nc.scalar.activation(at[:, 0:TC], prg[:], AF.Tanh,
                                         bias=gv(gc, 2), scale=0.5)
                    xt_ = rb.tile([128, TC], BF16, tag="rb", name="xt_")
                    nc.scalar.activation(xt_[:], pig[:], AF.Tanh,
                                         bias=gv(gc, 1), scale=0.5)
                    nc.vector.scalar_tensor_tensor(
                        xt_[:], xt_[:], 1.0, u_c[gc][:], OP.add, OP.mult)
                    A_t[gc] = at
                    X_t[gc] = xt_

            def emit_2b(gcs, A_t, X_t):
                n = len(gcs)
                # one wide fp32 tile per batch: alpha^2 slices land here, and
                # the affine + sqrt run once over the whole batch, so each
                # batch costs a single sqrt-table load instead of one per
                # channel chunk. fp32 because near-1 alpha^2 would round to
                # 1.0 in bf16 and collapse beta for slow channels.
                b2 = pb2.tile([128, 3 * TC], F32, tag="b2", name="b2")
                for i, gc in enumerate(gcs):
                    at = A_t[gc]
                    # alpha (in-place over tanh_rg), then alpha^2 = Square
                    nc.scalar.activation(at[:, 0:TC], at[:, 0:TC], AF.Exp,
                                         bias=gv(gc, 0), scale=gv(gc, 0))
                    nc.scalar.activation(b2[:, i * TC:(i + 1) * TC],
                                         at[:, 0:TC], AF.Square)
                # 0.25*(1 - alpha^2 + 1e-6): the 0.25 folds the two 0.5
                # factors from the tanh-sigmoid identities into beta
                nc.vector.tensor_scalar(b2[:, 0:n * TC], b2[:, 0:n * TC],
                                        -0.25, 0.25000025, OP.mult, OP.add)
                nc.scalar.activation(b2[:, 0:n * TC], b2[:, 0:n * TC],
                                     AF.Sqrt)
                for i, gc in enumerate(gcs):
                    at, xt_ = A_t[gc], X_t[gc]
                    # plain tensor_tensor is Pool-legal; keeps DVE free for
                    # the scans it feeds
                    nc.gpsimd.tensor_tensor(xt_[:], xt_[:],
                                            b2[:, i * TC:(i + 1) * TC],
                                            OP.mult)
                    hl = phl.tile([128, TC], BF16, tag="hl", name="hl")
                    nc.vector.tensor_tensor_scan(
                        hl[:], at[:, 0:TC], xt_[:], 0.0, OP.mult, OP.add)
                    pt_ = ppt.tile([128, TC], BF16, tag="pt", name="pt_")
                    nc.vector.tensor_tensor_scan(
                        pt_[:], at[:, 0:TC], at[:, 0:TC], 1.0,
                        OP.mult, OP.bypass)
                    nc.vector.tensor_copy(S_loc[:, gc:gc + 1],
                                          pt_[:, TC - 1:TC])
                    nc.vector.tensor_copy(S_loc[:, 12 + gc:13 + gc],
                                          hl[:, TC - 1:TC])
                    h_loc[gc] = hl
                    p_t[gc] = pt_

            def collective(g0, g1, coff):
                w = g1 - g0
                cin = dram.tile([128, 2 * w], F32, tag=f"cin{g0}",
                                name=f"cin{g0}")
                cout = dram.tile([4, 128, 2 * w], F32, tag=f"cout{g0}",
                                 name=f"cout{g0}")
                nc.sync.dma_start(cin[:, 0:w], S_loc[:, g0:g1])
                nc.sync.dma_start(cin[:, w:2 * w], S_loc[:, 12 + g0:12 + g1])
                nc.gpsimd.collective_compute(
                    "AllGather", OP.bypass,
                    replica_groups=[[0, 1, 2, 3], [4, 5, 6, 7]],
                    ins=[cin.opt()], outs=[cout.opt()])
                G = cst.tile([128, 8 * w], F32, tag=f"g{g0}", name=f"g{g0}")
                for r in range(4):
                    nc.sync.dma_start(G[:, r * 2 * w:(r + 1) * 2 * w], cout[r])
                # carry: p1 = b_0; p2 = A_1*p1 + b_1; p3 = A_2*p2 + b_2;
                # c = sel_1*p1 + sel_2*p2 + sel_3*p3 (sel is one-hot by core)
                p1 = G[:, w:2 * w]
                p2 = p2_t[:, coff:coff + w]
                p3 = p3_t[:, coff:coff + w]
                cc_ = c_t[:, coff:coff + w]
                nc.vector.tensor_tensor(p2, G[:, 2 * w:3 * w], p1, OP.mult)
                nc.vector.tensor_tensor(p2, p2, G[:, 3 * w:4 * w], OP.add)
                nc.vector.tensor_tensor(p3, G[:, 4 * w:5 * w], p2, OP.mult)
                nc.vector.tensor_tensor(p3, p3, G[:, 5 * w:6 * w], OP.add)
                nc.vector.tensor_scalar(cc_, p1, sel_t[:, 0:1], None, OP.mult)
                nc.vector.scalar_tensor_tensor(cc_, p2, sel_t[:, 1:2],
                                               cc_, OP.mult, OP.add)
                nc.vector.scalar_tensor_tensor(cc_, p3, sel_t[:, 2:3],
                                               cc_, OP.mult, OP.add)

            def emit_qw(gc):
                # q = gg*hl (into hl), w = gg*P (into gg); both carry-free.
                # Pinned to alternating engines so DVE and Pool split them.
                eng = nc.gpsimd if gc % 2 == 0 else nc.vector
                eng.tensor_tensor(h_loc[gc][:], gg_t[gc][:],
                                  h_loc[gc][:], OP.mult)
                eng.tensor_tensor(gg_t[gc][:], gg_t[gc][:],
                                  p_t[gc][:], OP.mult)

            def emit_gh(gc):
                # gh = w*c + q  (the only op that waits on the carry).
                # Per-partition-scalar ops are DVE-only in the V3 ISA.
                nc.vector.scalar_tensor_tensor(
                    gg_t[gc][:], gg_t[gc][:], c_t[:, gc:gc + 1],
                    h_loc[gc][:], OP.mult, OP.add)

            NA = 9  # pass-A channel count for the split output projection
            batches = [range(0, 3), range(3, 6), range(6, 9),
                       range(9, 11), range(11, 12)]
            state: list = [({}, {}) for _ in batches]
            emit_2a(batches[0], *state[0])
            emit_2a(batches[1], *state[1])
            emit_2b(batches[0], *state[0])
            emit_2a(batches[2], *state[2])
            emit_2b(batches[1], *state[1])
            emit_2a(batches[3], *state[3])
            emit_2b(batches[2], *state[2])
            collective(0, 9, 0)
            for gc in range(N25_EARLY):
                emit_qw(gc)
                emit_gh(gc)
            # channels 4..8: carry is known (R1), so the whole chain can
            # overlap the remaining gate matmuls
            for i in range(N25_EARLY, 9):
                gate_proj(i)
                emit_qw(i)
                emit_gh(i)
            emit_2a(batches[4], *state[4])
            emit_2b(batches[3], *state[3])
            emit_2b(batches[4], *state[4])
            collective(9, 12, 9)
            # pass-B output weights: one early DMA batch so the loads are
            # long done when pass B starts
            wtB = pwo.tile([128, NDC * (NH - NA) * 128], BF16, tag="woB",
                           name="woB", bufs=1)
            for dc in range(NDC):
                nc.sync.dma_start(
                    wtB[:, dc * 384:(dc + 1) * 384],
                    wout[dc, :, NA * 128:NH * 128])
            for i in (9, 10, 11):
                gate_proj(i)
                emit_qw(i)
                emit_gh(i)

            # ---- phase 3: output projection, two passes ----
            # Pass A accumulates channels 0..8, whose carries arrived with
            # the first AllGather mid-phase-2 — ~30us of PE work that hides
            # the second AllGather's fixed latency. Pass B adds channels
            # 9..11 on top. Partial sums park in the (dead) u_c ring as
            # bf16, costing no extra SBUF.
            otA_t = []
            for dc in range(NDC):
                wt = pwo.tile([128, NA * 128], BF16, tag="woA", name="woA",
                              bufs=3)
                nc.sync.dma_start(wt[:], wout[dc, :, 0:NA * 128])
                po = ps.tile([128, TC], F32, tag="ps", name="po")
                for gc in range(NA):
                    for th in range(2):
                        nc.tensor.matmul(
                            po[:, th * 512:(th + 1) * 512],
                            wt[:, gc * 128:(gc + 1) * 128],
                            gg_t[gc][:, th * 512:(th + 1) * 512],
                            start=(gc == 0), stop=(gc == NA - 1))
                otA = puc.tile([128, TC], BF16, tag="uc", name="otA")
                nc.scalar.copy(otA[:], po[:])
                otA_t.append(otA)

            for dc in range(NDC):
                ot = rf.tile([128, TPAD], F32, tag="rf", name="ot")
                for th in range(2):
                    sl = slice(th * 512, (th + 1) * 512)
                    # independent 1-bank psum per half so the th=1 matmuls
                    # never serialize against th=0's add
                    ph = psb.tile([128, 512], F32, tag="psb", name="pb")
                    for gc in range(NA, NH):
                        nc.tensor.matmul(
                            ph[:, 0:512],
                            wtB[:, dc * 384 + (gc - NA) * 128:
                                 dc * 384 + (gc - NA + 1) * 128],
                            gg_t[gc][:, sl],
                            start=(gc == NA), stop=(gc == NH - 1))
                    # final half: partial-A + partial-B, then store
                    # (gpsimd cannot read PSUM, so the adds stay on DVE)
                    nc.vector.tensor_tensor(ot[:, sl], otA_t[dc][:, sl],
                                            ph[:, 0:512], OP.add)
                    nc.sync.dma_start(out[dc, :, sl], ot[:, sl])

    nc.compile()
    return nc


def _softplus64(x):
    x = np.asarray(x, np.float64)
    return np.log1p(np.exp(-np.abs(x))) + np.maximum(x, 0.0)


def _prepare(x, W_proj, conv_w, conv_b, W_in, b_in, W_gate, b_gate,
             forget_lambda, W_out):
    x = np.asarray(x, np.float32)
    W_proj = np.asarray(W_proj, np.float32)
    conv_w = np.asarray(conv_w, np.float32)
    conv_b = np.asarray(conv_b, np.float32)
    W_in = np.asarray(W_in, np.float32)
    b_in = np.asarray(b_in, np.float32)
    W_gate = np.asarray(W_gate, np.float32)
    b_gate = np.asarray(b_gate, np.float32)
    forget_lambda = np.asarray(forget_lambda, np.float32)
    W_out = np.asarray(W_out, np.float32)

    # wproj[mc][k, c*128+m] = W_proj[row(mc)*128+m, c*128+k]
    # mc 0..11 = u rows (1536:3072), mc 12..23 = gate rows (0:1536)
    wp_ = W_proj.reshape(24, 128, NDC, 128).transpose(0, 3, 2, 1)
    order = list(range(12, 24)) + list(range(0, 12))
    wproj = np.ascontiguousarray(
        wp_[order].reshape(24, 128, NDC * 128)).astype(NPBF16)

    win_ = W_in.reshape(NH, 128, NH, 128).transpose(0, 3, 2, 1)
    wgt_ = W_gate.reshape(NH, 128, NH, 128).transpose(0, 3, 2, 1)
    wgates = np.ascontiguousarray(
        np.stack([win_, wgt_]).reshape(2, NH, 128, NH * 128)).astype(NPBF16)

    wout = np.ascontiguousarray(
        W_out.reshape(NDC, 128, NH, 128).transpose(0, 3, 2, 1)
        .reshape(NDC, 128, NH * 128)).astype(NPBF16)

    negrate = (-C_CONST * _softplus64(forget_lambda)).astype(np.float32)

    consts_base = np.zeros((128, NCC), np.float32)
    for hc in range(NH):
        sl = slice(hc * 128, (hc + 1) * 128)
        for k in range(KCONV):
            consts_base[:, hc * 5 + k] = conv_w[sl, 0, k]
        consts_base[:, hc * 5 + 4] = conv_b[sl]
        base = GV_BASE + hc * 3
        consts_base[:, base + 0] = 0.5 * negrate[sl]
        consts_base[:, base + 1] = 0.5 * b_in[sl]
        consts_base[:, base + 2] = 0.5 * b_gate[sl]

    in_maps = []
    for c in range(NCORE):
        bb, j = divmod(c, 4)
        lo = j * TC - PADL
        if lo < 0:
            chunk = np.concatenate(
                [np.zeros((PADL, DIM), np.float32), x[bb, 0:(j + 1) * TC]])
        else:
            chunk = x[bb, lo:(j + 1) * TC]
        xtc = np.ascontiguousarray(chunk.T).reshape(
            NDC, 128, TPAD).astype(NPBF16)
        consts = consts_base.copy()
        if j > 0:
            consts[:, SEL_BASE + j - 1] = 1.0
        in_maps.append({
            "xt": xtc, "wproj": wproj, "wgates": wgates, "wout": wout,
            "consts": consts,
        })
    return in_maps


def _get_nc():
    if "nc" not in _CACHE:
        _CACHE["nc"] = _build()
    return _CACHE["nc"]


def kernel(x, W_proj, conv_w, conv_b, W_in, b_in, W_gate, b_gate,
           forget_lambda, W_out):
    nc = _get_nc()
    in_maps = _prepare(x, W_proj, conv_w, conv_b, W_in, b_in, W_gate, b_gate,
                       forget_lambda, W_out)
    res = run_bass_kernel_spmd(nc, in_maps, core_ids=list(range(NCORE)))
    out = np.empty((B, T, DIM), np.float32)
    for c in range(NCORE):
        bb, j = divmod(c, 4)
        o = res.results[c]["out"].reshape(DIM, TC)
        out[bb, j * TC:(j + 1) * TC, :] = o.T
    return out


# revision 53
# speedup vs baseline: 1.0043x; 1.0043x over previous
"""Hawk (RG-LRU recurrent block) Trainium2 kernel, 8-core SPMD.

Sharding: data-parallel over B (2 groups of 4 cores) x sequence-parallel over T
(4 chunks of 1024 tokens per batch element). The diagonal linear recurrence
h_t = a_t * h_{t-1} + b_t is computed chunk-locally with the hardware
tensor_tensor_scan instruction, then stitched across cores with AllGathers of
per-chunk scan summaries (A = prod a, b = local final h) and the per-core
correction h = h_local + cumprod(a) * carry.

Layout on device: hidden channels on partitions, time on the free dimension.
All matmuls run in bf16 (both operands); the recurrence (alpha/beta/scan)
stays fp32. Weights are fully pre-transposed on the host so every weight DMA
is a dense contiguous copy, and the weight pool is deep enough to prefetch
across phase boundaries. The cumprod tiles stay SBUF-resident in bf16 (the
scan state is fp32 internally regardless of output dtype).

Key scheduling structure:
- sigmoid is computed as 0.5 + 0.5*tanh(x/2): Tanh/Exp/Square live in ONE
  activation-table set, so the per-channel chain costs 2 table loads
  (exp-set, sqrt-set) instead of 3.
- the summary AllGather is split in two (channels 0-8 mid-phase-2, 9-11 at
  the end) so most carry corrections overlap the gate matmuls.
- the correction is folded into the output operand as
  gh = (gg*hl) + (gg*P)*c: the two products need no carry and pipeline
  early; only one scalar_tensor_tensor per channel remains after the
  gather.
- the gate-half projection writes two half-width PSUM tiles from a separate
  pool, decoupling it from the gate-matmul PSUM ring.

Per core:
  phase 1: proj (u half) from xT -> u_pre -> causal depthwise conv -> u_c
           + 4 gate-half proj iterations to cover the conv's DVE tail
  phase 2: ig/rg gate matmuls -> tanh -> alpha/xbeta -> scans (h_loc, P),
           with the first summary AllGather after channel chunk 8
  phase 2.5: remaining 8 gate-half proj + gelu + gg*hl / gg*P products
  phase 3: per-channel carry fold + output projection
"""
import os

os.environ.setdefault("JAX_COMPILATION_CACHE_DIR", "/tmp/jax_cache_hawk")

import ml_dtypes
import numpy as np

import concourse.bacc as bacc
import concourse.mybir as mybir
import concourse.tile as tile
from concourse.bass_utils import run_bass_kernel_spmd

F32 = mybir.dt.float32
BF16 = mybir.dt.bfloat16
AF = mybir.ActivationFunctionType
OP = mybir.AluOpType
NPBF16 = ml_dtypes.bfloat16

DIM = 1024
HID = 1536
KCONV = 4
B = 2
T = 4096
C_CONST = 8.0

NCORE = 8
TC = T // 4          # tokens per core
NH = HID // 128      # 12 hidden chunks
NDC = DIM // 128     # 8 dim chunks
PADL = 4             # left pad columns (3 conv-overlap tokens + 1 alignment)
TPAD = TC + PADL     # 1028
GV_BASE = NH * 5     # packed consts: conv(60) | hn,hbi,hbg(36) | sel(3)
SEL_BASE = GV_BASE + NH * 3
NCC = SEL_BASE + 3

N25_EARLY = 4        # gate-proj iterations covering the phase-1 conv tail

_CACHE: dict = {}


def _build():
    nc = bacc.Bacc("TRN2", target_bir_lowering=False, debug=False,
                   num_devices=NCORE, dynamic_dma_scratch_size=8192)

    xt = nc.dram_tensor("xt", [NDC, 128, TPAD], BF16, kind="ExternalInput").ap()
    wproj = nc.dram_tensor("wproj", [24, 128, NDC * 128], BF16,
                           kind="ExternalInput").ap()
    wgates = nc.dram_tensor("wgates", [2, NH, 128, NH * 128], BF16,
                            kind="ExternalInput").ap()
    wout = nc.dram_tensor("wout", [NDC, 128, NH * 128], BF16,
                          kind="ExternalInput").ap()
    consts = nc.dram_tensor("consts", [128, NCC], F32, kind="ExternalInput").ap()
    out = nc.dram_tensor("out", [NDC, 128, TC], F32, kind="ExternalOutput").ap()

    with tile.TileContext(nc) as tc:
        with (
            tc.tile_pool(name="cst", bufs=1) as cst,
            tc.tile_pool(name="wp", bufs=10) as wp,
            tc.tile_pool(name="pxt", bufs=8) as pxt,
            tc.tile_pool(name="puc", bufs=12) as puc,
            tc.tile_pool(name="phl", bufs=12) as phl,
            tc.tile_pool(name="ppt", bufs=12) as ppt,
            tc.tile_pool(name="pgg", bufs=12) as pgg,
            tc.tile_pool(name="rf", bufs=5) as rf,
            tc.tile_pool(name="rb", bufs=6) as rb,
            tc.tile_pool(name="pb2", bufs=2) as pb2,
            tc.tile_pool(name="pwo", bufs=2) as pwo,
            tc.tile_pool(name="ps", bufs=3, space="PSUM") as ps,
            tc.tile_pool(name="psb", bufs=2, space="PSUM") as psb,
            tc.tile_pool(name="dram", bufs=1, space="DRAM") as dram,
        ):
            def load_w(src, ncols):
                wt = wp.tile([128, NH * 128], BF16, tag="w", name="wt")
                nc.sync.dma_start(wt[:, 0:ncols], src)
                return wt

            # first weight + x resident (emission order = DMA issue order)
            w_first = load_w(wproj[0], NDC * 128)
            xt_t = []
            for cc in range(NDC):
                x1 = pxt.tile([128, TPAD], BF16, tag="xt", name="x1")
                nc.sync.dma_start(x1[:], xt[cc])
                xt_t.append(x1)
            cs = cst.tile([128, NCC], F32, tag="cs", name="cs")
            nc.sync.dma_start(cs[:], consts[:])

            def cw(hc, k):
                return cs[:, hc * 5 + k: hc * 5 + k + 1]

            def gv(hc, k):
                return cs[:, GV_BASE + hc * 3 + k: GV_BASE + hc * 3 + k + 1]

            sel_t = cs[:, SEL_BASE: SEL_BASE + 3]

            S_loc = cst.tile([128, 24], F32, tag="sloc", name="S_loc")
            p2_t = cst.tile([128, NH], F32, tag="p2", name="p2_t")
            p3_t = cst.tile([128, NH], F32, tag="p3", name="p3_t")
            c_t = cst.tile([128, NH], F32, tag="ct", name="c_t")

            def proj_matmuls(pt, wt, prefix_pt, rhs_tiles=None):
                # prefix matmuls are interleaved per contraction chunk: a
                # standalone block of tiny N=4 matmuls would serialize on
                # unhidden LDWEIGHTS (not modeled, but real); sandwiched
                # between the 512-wide matmuls the loads overlap streaming
                src = rhs_tiles or xt_t
                for cc in range(NDC):
                    wslice = wt[:, cc * 128:(cc + 1) * 128]
                    for th in range(2):
                        nc.tensor.matmul(
                            pt[:, th * 512:(th + 1) * 512],
                            wslice,
                            src[cc][:, PADL + th * 512: PADL + (th + 1) * 512],
                            start=(cc == 0), stop=(cc == NDC - 1))
                    if prefix_pt is not None:
                        nc.tensor.matmul(
                            prefix_pt[:, 0:PADL],
                            wslice,
                            src[cc][:, 0:PADL],
                            start=(cc == 0), stop=(cc == NDC - 1))

            # ---- phase 1: u half of proj + causal conv ----
            u_c = []
            for mc in range(NH):
                wt = w_first if mc == 0 else load_w(wproj[mc], NDC * 128)
                pt = ps.tile([128, TC], F32, tag="ps", name="pt")
                p3 = psb.tile([128, 512], F32, tag="psb", name="p3")
                proj_matmuls(pt, wt, p3)
                upre = rf.tile([128, TPAD], F32, tag="rf", name="upre")
                nc.scalar.copy(upre[:, PADL:TPAD], pt[:])
                nc.vector.tensor_copy(upre[:, 0:PADL], p3[:, 0:PADL])
                acc = rf.tile([128, TPAD], F32, tag="rf", name="acc")
                # u_c[t] = sum_k w_k * u_pre[t-3+k] + conv_b; token t at col
                # PADL+t, so tap k reads cols [1+k : 1+k+TC)
                nc.vector.tensor_scalar(
                    acc[:, 0:TC], upre[:, 1:1 + TC], cw(mc, 0), cw(mc, 4),
                    OP.mult, OP.add)
                for k in (1, 2):
                    nc.vector.scalar_tensor_tensor(
                        acc[:, 0:TC], upre[:, 1 + k:1 + k + TC], cw(mc, k),
                        acc[:, 0:TC], OP.mult, OP.add)
                uc = puc.tile([128, TC], BF16, tag="uc", name="uc")
                nc.vector.scalar_tensor_tensor(
                    uc[:], upre[:, PADL:PADL + TC], cw(mc, 3),
                    acc[:, 0:TC], OP.mult, OP.add)
                u_c.append(uc)

            # ---- gate half of proj + gelu (two half-width PSUM tiles from
            # the psb pool so these matmuls never contend with the ps ring) --
            gg_t: list = [None] * NH

            def gate_proj(i):
                wt = load_w(wproj[NH + i], NDC * 128)
                gg = pgg.tile([128, TC], BF16, tag="gg", name="gg")
                phs = [psb.tile([128, 512], F32, tag="psb", name="ph")
                       for _ in range(2)]
                for cc in range(NDC):
                    for th in range(2):
                        nc.tensor.matmul(
                            phs[th][:, 0:512],
                            wt[:, cc * 128:(cc + 1) * 128],
                            xt_t[cc][:, PADL + th * 512: PADL + (th + 1) * 512],
                            start=(cc == 0), stop=(cc == NDC - 1))
                for th in range(2):
                    nc.scalar.activation(gg[:, th * 512:(th + 1) * 512],
                                         phs[th][:, 0:512], AF.Gelu)
                gg_t[i] = gg

            for i in range(N25_EARLY):
                gate_proj(i)

            # ---- phase 2: gates + scans, in batches ----
            # sigmoid(z) = 0.5 + 0.5*tanh(z/2), so alpha folds into one Exp:
            # alpha = exp(negrate*sig(rg)) = exp(hn*tanh_rg + hn), hn = negrate/2
            # and xbeta = beta*sig(ig)*u = sqrt(0.25*(1-alpha^2+eps)) *
            #             ((tanh_ig + 1)*u)
            h_loc: list = [None] * NH
            p_t: list = [None] * NH

            def emit_2a(gcs, A_t, X_t):
                for gc in gcs:
                    pig = ps.tile([128, TC], F32, tag="ps", name="pig")
                    prg = ps.tile([128, TC], F32, tag="ps", name="prg")
                    for dst, wi in ((pig, 0), (prg, 1)):
                        wt = load_w(wgates[wi, gc], NH * 128)
                        # hc-outer so each weight block feeds back-to-back
                        # matmuls (halves real LDWEIGHTS traffic)
                        for hc in range(NH):
                            for th in range(2):
                                nc.tensor.matmul(
                                    dst[:, th * 512:(th + 1) * 512],
                                    wt[:, hc * 128:(hc + 1) * 128],
                                    u_c[hc][:, th * 512:(th + 1) * 512],
                                    start=(hc == 0), stop=(hc == NH - 1))
                    at = rf.tile([128, TPAD], F32, tag="rf", name="at")
                    nc.scalar.activation(at[:, 0:TC], prg[:], AF.Tanh,
                                         bias=gv(gc, 2), scale=0.5)
                    xt_ = rb.tile([128, TC], BF16, tag="rb", name="xt_")
                    nc.scalar.activation(xt_[:], pig[:], AF.Tanh,
                                         bias=gv(gc, 1), scale=0.5)
                    nc.vector.scalar_tensor_tensor(
                        xt_[:], xt_[:], 1.0, u_c[gc][:], OP.add, OP.mult)
                    A_t[gc] = at
                    X_t[gc] = xt_

            def emit_2b(gcs, A_t, X_t):
                n = len(gcs)
                # one wide fp32 tile per batch: alpha^2 slices land here, and
                # the affine + sqrt run once over the whole batch, so each
                # batch costs a single sqrt-table load instead of one per
                # channel chunk. fp32 because near-1 alpha^2 would round to
                # 1.0 in bf16 and collapse beta for slow channels.
                b2 = pb2.tile([128, 3 * TC], F32, tag="b2", name="b2")
                for i, gc in enumerate(gcs):
                    at = A_t[gc]
                    # alpha (in-place over tanh_rg), then alpha^2 = Square
                    nc.scalar.activation(at[:, 0:TC], at[:, 0:TC], AF.Exp,
                                         bias=gv(gc, 0), scale=gv(gc, 0))
                    nc.scalar.activation(b2[:, i * TC:(i + 1) * TC],
                                         at[:, 0:TC], AF.Square)
                # 0.25*(1 - alpha^2 + 1e-6): the 0.25 folds the two 0.5
                # factors from the tanh-sigmoid identities into beta
                nc.vector.tensor_scalar(b2[:, 0:n * TC], b2[:, 0:n * TC],
                                        -0.25, 0.25000025, OP.mult, OP.add)
                nc.scalar.activation(b2[:, 0:n * TC], b2[:, 0:n * TC],
                                     AF.Sqrt)
                for i, gc in enumerate(gcs):
                    at, xt_ = A_t[gc], X_t[gc]
                    # plain tensor_tensor is Pool-legal; keeps DVE free for
                    # the scans it feeds
                    nc.gpsimd.tensor_tensor(xt_[:], xt_[:],
                                            b2[:, i * TC:(i + 1) * TC],
                                            OP.mult)
                    hl = phl.tile([128, TC], BF16, tag="hl", name="hl")
                    nc.vector.tensor_tensor_scan(
                        hl[:], at[:, 0:TC], xt_[:], 0.0, OP.mult, OP.add)
                    pt_ = ppt.tile([128, TC], BF16, tag="pt", name="pt_")
                    nc.vector.tensor_tensor_scan(
                        pt_[:], at[:, 0:TC], at[:, 0:TC], 1.0,
                        OP.mult, OP.bypass)
                    nc.vector.tensor_copy(S_loc[:, gc:gc + 1],
                                          pt_[:, TC - 1:TC])
                    nc.vector.tensor_copy(S_loc[:, 12 + gc:13 + gc],
                                          hl[:, TC - 1:TC])
                    h_loc[gc] = hl
                    p_t[gc] = pt_

            def collective(g0, g1, coff):
                w = g1 - g0
                cin = dram.tile([128, 2 * w], F32, tag=f"cin{g0}",
                                name=f"cin{g0}")
                cout = dram.tile([4, 128, 2 * w], F32, tag=f"cout{g0}",
                                 name=f"cout{g0}")
                nc.sync.dma_start(cin[:, 0:w], S_loc[:, g0:g1])
                nc.sync.dma_start(cin[:, w:2 * w], S_loc[:, 12 + g0:12 + g1])
                nc.gpsimd.collective_compute(
                    "AllGather", OP.bypass,
                    replica_groups=[[0, 1, 2, 3], [4, 5, 6, 7]],
                    ins=[cin.opt()], outs=[cout.opt()])
                G = cst.tile([128, 8 * w], F32, tag=f"g{g0}", name=f"g{g0}")
                for r in range(4):
                    nc.sync.dma_start(G[:, r * 2 * w:(r + 1) * 2 * w], cout[r])
                # carry: p1 = b_0; p2 = A_1*p1 + b_1; p3 = A_2*p2 + b_2;
                # c = sel_1*p1 + sel_2*p2 + sel_3*p3 (sel is one-hot by core)
                p1 = G[:, w:2 * w]
                p2 = p2_t[:, coff:coff + w]
                p3 = p3_t[:, coff:coff + w]
                cc_ = c_t[:, coff:coff + w]
                nc.vector.tensor_tensor(p2, G[:, 2 * w:3 * w], p1, OP.mult)
                nc.vector.tensor_tensor(p2, p2, G[:, 3 * w:4 * w], OP.add)
                nc.vector.tensor_tensor(p3, G[:, 4 * w:5 * w], p2, OP.mult)
                nc.vector.tensor_tensor(p3, p3, G[:, 5 * w:6 * w], OP.add)
                nc.vector.tensor_scalar(cc_, p1, sel_t[:, 0:1], None, OP.mult)
                nc.vector.scalar_tensor_tensor(cc_, p2, sel_t[:, 1:2],
                                               cc_, OP.mult, OP.add)
                nc.vector.scalar_tensor_tensor(cc_, p3, sel_t[:, 2:3],
                                               cc_, OP.mult, OP.add)

            def emit_qw(gc):
                # q = gg*hl (into hl), w = gg*P (into gg); both carry-free.
                # Pinned to alternating engines so DVE and Pool split them.
                eng = nc.gpsimd if gc % 2 == 0 else nc.vector
                eng.tensor_tensor(h_loc[gc][:], gg_t[gc][:],
                                  h_loc[gc][:], OP.mult)
                eng.tensor_tensor(gg_t[gc][:], gg_t[gc][:],
                                  p_t[gc][:], OP.mult)

            def emit_gh(gc):
                # gh = w*c + q  (the only op that waits on the carry).
                # Per-partition-scalar ops are DVE-only in the V3 ISA.
                nc.vector.scalar_tensor_tensor(
                    gg_t[gc][:], gg_t[gc][:], c_t[:, gc:gc + 1],
                    h_loc[gc][:], OP.mult, OP.add)

            NA = 9  # pass-A channel count for the split output projection
            batches = [range(0, 3), range(3, 6), range(6, 9),
                       range(9, 11), range(11, 12)]
            state: list = [({}, {}) for _ in batches]
            emit_2a(batches[0], *state[0])
            emit_2a(batches[1], *state[1])
            emit_2b(batches[0], *state[0])
            emit_2a(batches[2], *state[2])
            emit_2b(batches[1], *state[1])
            emit_2a(batches[3], *state[3])
            emit_2b(batches[2], *state[2])
            collective(0, 9, 0)
            for gc in range(N25_EARLY):
                emit_qw(gc)
                emit_gh(gc)
            # channels 4..8: carry is known (R1), so the whole chain can
            # overlap the remaining gate matmuls
            for i in range(N25_EARLY, 9):
                gate_proj(i)
                emit_qw(i)
                emit_gh(i)
            emit_2a(batches[4], *state[4])
            emit_2b(batches[3], *state[3])
            emit_2b(batches[4], *state[4])
            collective(9, 12, 9)
            # pass-B output weights: one early DMA batch so the loads are
            # long done when pass B starts
            wtB = pwo.tile([128, NDC * (NH - NA) * 128], BF16, tag="woB",
                           name="woB", bufs=1)
            for dc in range(NDC):
                nc.sync.dma_start(
                    wtB[:, dc * 384:(dc + 1) * 384],
                    wout[dc, :, NA * 128:NH * 128])
            for i in (9, 10, 11):
                gate_proj(i)
                emit_qw(i)
                emit_gh(i)

            # ---- phase 3: output projection, two passes ----
            # Pass A accumulates channels 0..8, whose carries arrived with
            # the first AllGather mid-phase-2 — ~30us of PE work that hides
            # the second AllGather's fixed latency. Pass B adds channels
            # 9..11 on top. Partial sums park in the (dead) u_c ring as
            # bf16, costing no extra SBUF.
            otA_t = []
            for dc in range(NDC):
                wt = pwo.tile([128, NA * 128], BF16, tag="woA", name="woA",
                              bufs=3)
                nc.sync.dma_start(wt[:], wout[dc, :, 0:NA * 128])
                po = ps.tile([128, TC], F32, tag="ps", name="po")
                for gc in range(NA):
                    for th in range(2):
                        nc.tensor.matmul(
                            po[:, th * 512:(th + 1) * 512],
                            wt[:, gc * 128:(gc + 1) * 128],
                            gg_t[gc][:, th * 512:(th + 1) * 512],
                            start=(gc == 0), stop=(gc == NA - 1))
                otA = puc.tile([128, TC], BF16, tag="uc", name="otA")
                nc.scalar.copy(otA[:], po[:])
                otA_t.append(otA)

            for dc in range(NDC):
                ot = rf.tile([128, TPAD], F32, tag="rf", name="ot")
                for th in range(2):
                    sl = slice(th * 512, (th + 1) * 512)
                    # independent psum per half so the th=1 matmuls never
                    # serialize against th=0's add; th=1 borrows the ps ring
                    # (idle after pass A) to deepen the pipeline
                    if th == 0:
                        ph = psb.tile([128, 512], F32, tag="psb", name="pb")
                    else:
                        ph = ps.tile([128, TC], F32, tag="ps", name="pb")
                    for gc in range(NA, NH):
                        nc.tensor.matmul(
                            ph[:, 0:512],
                            wtB[:, dc * 384 + (gc - NA) * 128:
                                 dc * 384 + (gc - NA + 1) * 128],
                            gg_t[gc][:, sl],
                            start=(gc == NA), stop=(gc == NH - 1))
                    # final half: partial-A + partial-B, then store
                    # (gpsimd cannot read PSUM, so the adds stay on DVE)
                    nc.vector.tensor_tensor(ot[:, sl], otA_t[dc][:, sl],
                                            ph[:, 0:512], OP.add)
                    nc.sync.dma_start(out[dc, :, sl], ot[:, sl])

    nc.compile()
    return nc


def _softplus64(x):
    x = np.asarray(x, np.float64)
    return np.log1p(np.exp(-np.abs(x))) + np.maximum(x, 0.0)


def _prepare(x, W_proj, conv_w, conv_b, W_in, b_in, W_gate, b_gate,
             forget_lambda, W_out):
    x = np.asarray(x, np.float32)
    W_proj = np.asarray(W_proj, np.float32)
    conv_w = np.asarray(conv_w, np.float32)
    conv_b = np.asarray(conv_b, np.float32)
    W_in = np.asarray(W_in, np.float32)
    b_in = np.asarray(b_in, np.float32)
    W_gate = np.asarray(W_gate, np.float32)
    b_gate = np.asarray(b_gate, np.float32)
    forget_lambda = np.asarray(forget_lambda, np.float32)
    W_out = np.asarray(W_out, np.float32)

    # wproj[mc][k, c*128+m] = W_proj[row(mc)*128+m, c*128+k]
    # mc 0..11 = u rows (1536:3072), mc 12..23 = gate rows (0:1536)
    wp_ = W_proj.reshape(24, 128, NDC, 128).transpose(0, 3, 2, 1)
    order = list(range(12, 24)) + list(range(0, 12))
    wproj = np.ascontiguousarray(
        wp_[order].reshape(24, 128, NDC * 128)).astype(NPBF16)

    win_ = W_in.reshape(NH, 128, NH, 128).transpose(0, 3, 2, 1)
    wgt_ = W_gate.reshape(NH, 128, NH, 128).transpose(0, 3, 2, 1)
    wgates = np.ascontiguousarray(
        np.stack([win_, wgt_]).reshape(2, NH, 128, NH * 128)).astype(NPBF16)

    wout = np.ascontiguousarray(
        W_out.reshape(NDC, 128, NH, 128).transpose(0, 3, 2, 1)
        .reshape(NDC, 128, NH * 128)).astype(NPBF16)

    negrate = (-C_CONST * _softplus64(forget_lambda)).astype(np.float32)

    consts_base = np.zeros((128, NCC), np.float32)
    for hc in range(NH):
        sl = slice(hc * 128, (hc + 1) * 128)
        for k in range(KCONV):
            consts_base[:, hc * 5 + k] = conv_w[sl, 0, k]
        consts_base[:, hc * 5 + 4] = conv_b[sl]
        base = GV_BASE + hc * 3
        consts_base[:, base + 0] = 0.5 * negrate[sl]
        consts_base[:, base + 1] = 0.5 * b_in[sl]
        consts_base[:, base + 2] = 0.5 * b_gate[sl]

    in_maps = []
    for c in range(NCORE):
        bb, j = divmod(c, 4)
        lo = j * TC - PADL
        if lo < 0:
            chunk = np.concatenate(
                [np.zeros((PADL, DIM), np.float32), x[bb, 0:(j + 1) * TC]])
        else:
            chunk = x[bb, lo:(j + 1) * TC]
        xtc = np.ascontiguousarray(chunk.T).reshape(
            NDC, 128, TPAD).astype(NPBF16)
        consts = consts_base.copy()
        if j > 0:
            consts[:, SEL_BASE + j - 1] = 1.0
        in_maps.append({
            "xt": xtc, "wproj": wproj, "wgates": wgates, "wout": wout,
            "consts": consts,
        })
    return in_maps


def _get_nc():
    if "nc" not in _CACHE:
        _CACHE["nc"] = _build()
    return _CACHE["nc"]


def kernel(x, W_proj, conv_w, conv_b, W_in, b_in, W_gate, b_gate,
           forget_lambda, W_out):
    nc = _get_nc()
    in_maps = _prepare(x, W_proj, conv_w, conv_b, W_in, b_in, W_gate, b_gate,
                       forget_lambda, W_out)
    res = run_bass_kernel_spmd(nc, in_maps, core_ids=list(range(NCORE)))
    out = np.empty((B, T, DIM), np.float32)
    for c in range(NCORE):
        bb, j = divmod(c, 4)
        o = res.results[c]["out"].reshape(DIM, TC)
        out[bb, j * TC:(j + 1) * TC, :] = o.T
    return out
